# revision 11
# baseline (speedup 1.0000x reference)
"""Trainium2 Bass kernel for DeterministicPhysicalLikelihoodBuilder.

Strategy (pure data-parallel over batch, 2 batches/core on 8 cores):
  - Stream [128t, 1025f] tiles of phase/comb; compute the weighted trough
    spectrum elementwise (ACT/DVE), with the row-sum fused into the final
    scalar_tensor_tensor op.
  - The einsums against the [D,F] basis (full-range + 4 subbands) are all
    partial sums of ONE matmul split at the subband boundaries along the
    contraction axis: PE-transpose trough tiles to [f,t] layout, then
    accumulate per-band PSUM tiles with K-sliced matmuls.
  - Normalization by mean(trough) is linear, so it is deferred to the
    PSUM->SBUF copies (per-partition scale on ACT).
  - Channels are assembled strided into a [128, 640] tile so both outputs
    stream out as fully contiguous DMA.
"""

import os
from contextlib import ExitStack

import numpy as np

B, T, F, D = 16, 2048, 1025, 64
S = 4
NCORES = 8
BPC = B // NCORES          # batches per core
P = 128
NT = T // P                # 16 tiles of 128 rows per batch
EPS = 1e-6
NCH = 10
SOUND_SPEED = 343.0

_PROG_CACHE = {}
LAST_RESULTS = None        # stashed BassKernelResults for test harness


def _band_cuts(freq):
    """Subband boundaries as f-indices [0, c1, c2, c3, F] (bands contiguous)."""
    edges = [float(freq.min()), 500.0, 2000.0, 8000.0, float(freq.max()) + 1.0]
    cuts = [0]
    for lo, hi in zip(edges[:-1], edges[1:]):
        idx = np.nonzero((freq >= lo) & (freq < hi))[0]
        assert idx.size > 0 and int(idx[0]) == cuts[-1] and np.all(np.diff(idx) == 1)
        cuts.append(int(idx[-1]) + 1)
    assert cuts[-1] == F
    return cuts


def _layout(cuts):
    """Band-aligned f repacking: each band padded to whole 128-blocks.

    Returns (nblk, fpad, spans) where spans[s] = (src_lo, src_hi, dst_lo,
    blk_lo, nblk_s): band s's source f range, its (128-aligned) destination
    column, and how many blocks it occupies.
    """
    spans = []
    blk = 0
    for s in range(4):
        lo, hi = cuts[s], cuts[s + 1]
        nb = -(-(hi - lo) // P)
        spans.append((lo, hi, blk * P, blk, nb))
        blk += nb
    return blk, blk * P, spans


def _build_program(cuts, denom):
    import concourse.bacc as bacc
    import concourse.tile as tile
    from concourse import masks, mybir

    dt = mybir.dt
    f32 = dt.float32
    AF = mybir.ActivationFunctionType
    ALU = mybir.AluOpType
    AX = mybir.AxisListType

    NBLK, FPAD, spans = _layout(cuts)

    nc = bacc.Bacc(
        "TRN2",
        target_bir_lowering=False,
        debug=False,
        enable_asserts=False,
        num_devices=NCORES,
    )

    ph_d = nc.dram_tensor("phase", [BPC, 1, T, F], f32, kind="ExternalInput").ap()
    cb_d = nc.dram_tensor("comb", [BPC, 2, T, F], f32, kind="ExternalInput").ap()
    sc_d = nc.dram_tensor("scalar", [BPC, T, S], f32, kind="ExternalInput").ap()
    ob_d = nc.dram_tensor("obs", [BPC, T, S], f32, kind="ExternalInput").ap()
    re_d = nc.dram_tensor("rel", [BPC, T, S], f32, kind="ExternalInput").ap()
    st_d = nc.dram_tensor("stpacc", [BPC, 1, T, D], f32, kind="ExternalInput").ap()
    bs_d = nc.dram_tensor("basisc", [P, NBLK * D], f32, kind="ExternalInput").ap()
    lik_d = nc.dram_tensor("lik", [BPC, T, D, NCH], f32, kind="ExternalOutput").ap()
    lg_d = nc.dram_tensor("logits", [BPC, T, D], f32, kind="ExternalOutput").ap()

    with tile.TileContext(nc) as tc, ExitStack() as ctx:
        const_pool = ctx.enter_context(tc.tile_pool(name="const", bufs=1))
        ident = const_pool.tile([P, P], f32, name="ident")
        masks.make_identity(nc, ident[:])
        basis_sb = const_pool.tile([P, NBLK * D], f32, name="basis_sb")
        nc.sync.dma_start(basis_sb[:], bs_d)

        inp = ctx.enter_context(tc.tile_pool(name="inp", bufs=3))
        work = ctx.enter_context(tc.tile_pool(name="work", bufs=2))
        small = ctx.enter_context(tc.tile_pool(name="small", bufs=3))
        outp = ctx.enter_context(tc.tile_pool(name="outp", bufs=3))
        tps = ctx.enter_context(tc.tile_pool(name="tps", bufs=3, space="PSUM"))
        bps = ctx.enter_context(tc.tile_pool(name="bps", bufs=2, space="PSUM"))

        for b in range(BPC):
            for i in range(NT):
                tsl = slice(i * P, (i + 1) * P)

                ph = inp.tile([P, F], f32, tag="ph")
                nc.sync.dma_start(ph[:], ph_d[b, 0, tsl, :])
                c0 = inp.tile([P, F], f32, tag="c0")
                nc.sync.dma_start(c0[:], cb_d[b, 0, tsl, :])
                c1 = inp.tile([P, F], f32, tag="c1")
                nc.sync.dma_start(c1[:], cb_d[b, 1, tsl, :])
                stp = small.tile([P, D], f32, tag="stp")
                nc.sync.dma_start(stp[:], st_d[b, 0, tsl, :])
                sct = small.tile([P, S], f32, tag="sct")
                nc.sync.dma_start(sct[:], sc_d[b, tsl, :])
                obt = small.tile([P, S], f32, tag="obt")
                nc.sync.dma_start(obt[:], ob_d[b, tsl, :])
                ret = small.tile([P, S], f32, tag="ret")
                nc.sync.dma_start(ret[:], re_d[b, tsl, :])

                # ---- trough spectrum (elementwise, [t, f] layout) ----
                msum = small.tile([P, 1], f32, tag="msum")
                nc.vector.tensor_reduce(msum[:], ph[:], AX.X, ALU.add)
                mrow = small.tile([P, 1], f32, tag="mrow")
                nc.vector.tensor_scalar_mul(mrow[:], msum[:], 1.0 / F)
                # trough = relu(mean - x)
                trough = work.tile([P, F], f32, tag="trough")
                nc.scalar.activation(trough[:], ph[:], AF.Relu, bias=mrow[:], scale=-1.0)
                a0 = work.tile([P, F], f32, tag="a0")
                nc.scalar.activation(a0[:], c0[:], AF.Abs, scale=0.25)
                a1 = work.tile([P, F], f32, tag="a1")
                nc.scalar.activation(a1[:], c1[:], AF.Abs)
                s_t = work.tile([P, F], f32, tag="s_t")
                nc.vector.tensor_add(s_t[:], a0[:], a1[:])
                # t2 = trough * (1 + |c1| + 0.25|c0|), written band-aligned
                # (each band zero-padded to whole 128-blocks) with fused
                # per-band row-sums.
                t2 = work.tile([P, FPAD], f32, tag="t2")
                t2rows = small.tile([P, 4], f32, tag="t2rows")
                for s, (lo, hi, dst, _, nb) in enumerate(spans):
                    n = hi - lo
                    nc.vector.scalar_tensor_tensor(
                        t2[:, dst:dst + n], s_t[:, lo:hi], 1.0, trough[:, lo:hi],
                        op0=ALU.add, op1=ALU.mult, accum_out=t2rows[:, s:s + 1],
                    )
                    pad = nb * P - n
                    if pad:
                        nc.gpsimd.memset(t2[:, dst + n:dst + nb * P], 0.0)
                t2row = small.tile([P, 1], f32, tag="t2row")
                nc.vector.tensor_reduce(t2row[:], t2rows[:], AX.X, ALU.add)

                # ---- PE transpose to [f, t] blocks ----
                ttr = work.tile([P, FPAD], f32, tag="ttr")
                for blk in range(NBLK):
                    pt = tps.tile([P, P], f32, tag="pt")
                    nc.tensor.transpose(pt[:], t2[:, blk * P:(blk + 1) * P], ident[:])
                    dst = ttr[:, blk * P:(blk + 1) * P]
                    if blk % 2 == 0:
                        nc.scalar.copy(dst, pt[:])
                    else:
                        nc.vector.tensor_copy(dst, pt[:])

                # ---- band-partial matmuls (accumulate over K blocks) ----
                pband = bps.tile([P, 4 * D], f32, tag="pband")
                for s, (_, _, _, blk_lo, nb) in enumerate(spans):
                    for j in range(nb):
                        blk = blk_lo + j
                        nc.tensor.matmul(
                            pband[:, s * D:(s + 1) * D],
                            ttr[:, blk * P:(blk + 1) * P],
                            basis_sb[:, blk * D:(blk + 1) * D],
                            start=(j == 0),
                            stop=(j == nb - 1),
                        )

                # ---- normalization scalars ----
                mx = small.tile([P, 1], f32, tag="mx")
                nc.vector.tensor_scalar(mx[:], t2row[:], 1.0 / F, EPS,
                                        op0=ALU.mult, op1=ALU.max)
                rc = small.tile([P, 1], f32, tag="rc")
                nc.vector.reciprocal(rc[:], mx[:])
                rcs = []
                for s in range(4):
                    r_ = small.tile([P, 1], f32, tag=f"rcs{s}", name=f"rcs{s}")
                    nc.vector.tensor_scalar_mul(r_[:], rc[:], 1.0 / denom[s])
                    rcs.append(r_)

                # ---- assemble likelihood channels [128, 64, 10] ----
                # band channels first (single PSUM operand per op); the
                # full-range channel 0 is then a weighted sum of them:
                # ch0 = sum_s denom_s*ch_s / F.
                L = outp.tile([P, D * NCH], f32, tag="L")
                Lv = L[:].rearrange("p (d c) -> p d c", c=NCH)
                for s in range(4):
                    nc.scalar.mul(Lv[:, :, 1 + s], pband[:, s * D:(s + 1) * D], rcs[s][:])
                z1 = small.tile([P, D], f32, tag="z1")
                nc.vector.scalar_tensor_tensor(
                    z1[:], Lv[:, :, 1], denom[0] / denom[1], Lv[:, :, 2],
                    op0=ALU.mult, op1=ALU.add)
                z2 = small.tile([P, D], f32, tag="z2")
                nc.vector.scalar_tensor_tensor(
                    z2[:], z1[:], denom[1] / denom[2], Lv[:, :, 3],
                    op0=ALU.mult, op1=ALU.add)
                z3 = small.tile([P, D], f32, tag="z3")
                nc.vector.scalar_tensor_tensor(
                    z3[:], z2[:], denom[2] / denom[3], Lv[:, :, 4],
                    op0=ALU.mult, op1=ALU.add)
                nc.scalar.mul(Lv[:, :, 0], z3[:], denom[3] / F)

                # stp channel
                str_ = small.tile([P, D], f32, tag="str_")
                ssum = small.tile([P, 1], f32, tag="ssum")
                nc.scalar.activation(str_[:], stp[:], AF.Relu, accum_out=ssum[:])
                smx = small.tile([P, 1], f32, tag="smx")
                nc.vector.tensor_scalar(smx[:], ssum[:], 1.0 / D, EPS,
                                        op0=ALU.mult, op1=ALU.max)
                src = small.tile([P, 1], f32, tag="src")
                nc.vector.reciprocal(src[:], smx[:])
                nc.scalar.mul(Lv[:, :, 5], str_[:], src[:])

                # quality / scalar channels (broadcast [P,1] across d)
                osum = small.tile([P, 1], f32, tag="osum")
                nc.vector.tensor_reduce(osum[:], obt[:], AX.X, ALU.add)
                omean = small.tile([P, 1], f32, tag="omean")
                nc.vector.tensor_scalar_mul(omean[:], osum[:], 1.0 / S)
                nc.scalar.activation(Lv[:, :, 6], str_[:], AF.Identity,
                                     bias=omean[:], scale=0.0)
                rsum = small.tile([P, 1], f32, tag="rsum")
                nc.vector.tensor_reduce(rsum[:], ret[:], AX.X, ALU.add)
                rmean = small.tile([P, 1], f32, tag="rmean")
                nc.vector.tensor_scalar_mul(rmean[:], rsum[:], 1.0 / S)
                nc.scalar.activation(Lv[:, :, 7], str_[:], AF.Identity,
                                     bias=rmean[:], scale=0.0)
                iss = small.tile([P, 1], f32, tag="iss")
                nc.vector.tensor_scalar(iss[:], sct[:, 0:1], 0.0, 1.0,
                                        op0=ALU.max, op1=ALU.min)
                nc.scalar.activation(Lv[:, :, 8], str_[:], AF.Identity,
                                     bias=iss[:], scale=0.0)
                ab = small.tile([P, 1], f32, tag="ab")
                nc.scalar.activation(ab[:], sct[:, 1:2], AF.Abs)
                rho = small.tile([P, 1], f32, tag="rho")
                nc.vector.tensor_scalar_min(rho[:], ab[:], 1.0)
                nc.scalar.activation(Lv[:, :, 9], str_[:], AF.Identity,
                                     bias=rho[:], scale=0.0)

                # ---- logits = mean(channels) * (0.5 + 0.5 * is_sound) ----
                graw = small.tile([P, D], f32, tag="graw")
                nc.vector.tensor_reduce(graw[:], Lv[:, :, :], AX.X, ALU.add)
                w = small.tile([P, 1], f32, tag="w")
                nc.vector.tensor_scalar(w[:], iss[:], 0.5 / NCH, 0.5 / NCH,
                                        op0=ALU.mult, op1=ALU.add)
                G = outp.tile([P, D], f32, tag="G")
                nc.scalar.mul(G[:], graw[:], w[:])

                nc.sync.dma_start(lik_d[b, tsl, :, :], Lv[:, :, :])
                nc.sync.dma_start(lg_d[b, tsl, :], G[:])

    nc.compile()
    return nc


def _get_program(cuts, denom):
    key = tuple(cuts)
    if key not in _PROG_CACHE:
        _PROG_CACHE[key] = _build_program(cuts, denom)
    return _PROG_CACHE[key]


def _host_basis(freq, spacing):
    pattern = 0.5 * (1.0 + np.cos(
        np.float32(2.0 * np.pi) * (freq[None, :] / np.maximum(spacing[:, None], np.float32(1e-6)))
    ).astype(np.float32))
    basis = pattern / np.maximum(pattern.mean(axis=-1, keepdims=True), np.float32(EPS))
    return basis.astype(np.float32)   # [D, F]


def _host_basisc(freq, spacing, cuts):
    """Basis repacked to the band-aligned [f, d] block layout the kernel uses:
    [P, NBLK*D] where block b occupies columns [b*D, (b+1)*D)."""
    basis = _host_basis(freq, spacing)       # [D, F]
    nblk, fpad, spans = _layout(cuts)
    bp = np.zeros((fpad, D), np.float32)
    for lo, hi, dst, _, _ in spans:
        bp[dst:dst + (hi - lo)] = basis.T[lo:hi]
    return np.ascontiguousarray(
        bp.reshape(nblk, P, D).transpose(1, 0, 2).reshape(P, nblk * D)
    )


def kernel(phase, comb, scalar, scalar_observed_mask, scalar_reliable_mask,
           stpacc, frequencies_hz, spacing_grid_hz):
    global LAST_RESULTS
    from concourse.bass_utils import run_bass_kernel_spmd

    phase = np.asarray(phase, dtype=np.float32)
    comb = np.asarray(comb, dtype=np.float32)
    scalar = np.asarray(scalar, dtype=np.float32)
    obs = np.asarray(scalar_observed_mask, dtype=np.float32)
    rel = np.asarray(scalar_reliable_mask, dtype=np.float32)
    stpacc = np.asarray(stpacc, dtype=np.float32)
    freq = np.asarray(frequencies_hz, dtype=np.float32)
    spacing = np.asarray(spacing_grid_hz, dtype=np.float32)

    cuts = _band_cuts(freq)
    denom = [float(max(cuts[s + 1] - cuts[s], 1)) for s in range(4)]
    nc = _get_program(cuts, denom)

    basisc = _host_basisc(freq, spacing, cuts)

    in_maps = []
    for c in range(NCORES):
        bsl = slice(c * BPC, (c + 1) * BPC)
        in_maps.append({
            "phase": np.ascontiguousarray(phase[bsl]),
            "comb": np.ascontiguousarray(comb[bsl]),
            "scalar": np.ascontiguousarray(scalar[bsl]),
            "obs": np.ascontiguousarray(obs[bsl]),
            "rel": np.ascontiguousarray(rel[bsl]),
            "stpacc": np.ascontiguousarray(stpacc[bsl]),
            "basisc": basisc,
        })

    trace = bool(int(os.environ.get("BASS_KERNEL_TRACE", "0")))
    res = run_bass_kernel_spmd(nc, in_maps, list(range(NCORES)), trace=trace)
    LAST_RESULTS = res

    lik = np.concatenate([res.results[c]["lik"] for c in range(NCORES)], axis=0)
    logits = np.concatenate([res.results[c]["logits"] for c in range(NCORES)], axis=0)

    dist = (100.0 * SOUND_SPEED) / (2.0 * np.maximum(spacing, np.float32(1e-6)))
    return (lik.astype(np.float32), logits.astype(np.float32),
            spacing.astype(np.float32), dist.astype(np.float32))
